# revision 6
# baseline (speedup 1.0000x reference)
"""Trainium2 Bass kernel for CustomPatchEmbedding — v3.

out[b,n,e] = sum_k patch(b,n)[k] * W[e,k] + bias[e]; patches are 16x16x3
windows of x at (start_h, start_w)[b,n].  8 NeuronCores, data-parallel over
batch (8 images/core, TOK=4608 tokens/core).

Pipeline per chunk of `nt` 128-token blocks:
 1. `nt` indirect row-gathers (one int32 element-offset per partition) land
    token 128*tb+p's patch as 768 contiguous fp16 at G[p, tb-slab] — the
    band layout xb makes every 16x16x3 patch contiguous.
 2. One dma_gather(transpose=True, SBUF source, tokens_per_rank=128) turns
    G into C[p, c, i] = patch element (128c + p) of token i: the matmul-
    ready [k, tok] layout (xbar transposes at 16-bit granularity).
 3. 6x6 weight-stationary fp16 matmuls: psum[e', i] += Wk[128m+p, 128e+e']
    * C[p, m, i], N = 128*nt columns per matmul.  PE does nothing else.
 4. psum -> SBUF drain with per-partition bias (ScalarE activation-Identity
    / VectorE tensor_scalar_add, alternating), then HWDGE DMA to the
    transposed DRAM output [E, TOK]; the host untransposes.

The output is computed transposed so tokens live on the matmul free axis
(weights stationary, loaded 36x per chunk but LDWEIGHTS pipelines under the
moving streams).  Small chunks at the head shorten the pipeline fill;
PE warmup matmuls bridge the fill window so real matmuls run at full
p-state from the first instruction.
"""
import numpy as np

import concourse.bass as bass
import concourse.bacc as bacc
import concourse.mybir as mybir
import concourse.tile as tile
from concourse.bass_utils import run_bass_kernel_spmd

B, C, H, W = 64, 3, 384, 384
N, E, P = 576, 768, 16
NCORES = 8
BPC = B // NCORES          # 8 images per core
TOK = BPC * N              # 4608 tokens per core
CPP = C * P * P            # 768 = contraction length
KC = CPP // 128            # 6 k-chunks
EC = E // 128              # 6 e-chunks
NB = W - P + 1             # 369 one-px-step bands
BW = P * C                 # 48 fp16 per band row
NTB = TOK // 128           # 36 token-blocks of 128
NWARM = 16
GP_BUFS = 8
CP_BUFS = 6
_POOLCFG = {}
CHUNKS = [(0, 1), (1, 1)] + [(2 + 2 * i, 2) for i in range(17)]
assert sum(n for _, n in CHUNKS) == NTB

f32 = mybir.dt.float32
fp16 = mybir.dt.float16
i32 = mybir.dt.int32
i16 = mybir.dt.int16

_cached = {}


def build_nc(debug=False, warm=NWARM, chunks=CHUNKS, out_halves=3,
             lookahead=2, pe_head=1):
    nc = bacc.Bacc(trn_type="TRN2", debug=debug)
    # xb band table: element (b*369+sw)*384*48 + sh*48 + j is
    # x[b, j%3, sh, sw + j//3] (HWC); every patch is 768 contiguous fp16.
    xb = nc.dram_tensor("xb", [1, BPC * NB * H * BW], fp16,
                        kind="ExternalInput")
    idx = nc.dram_tensor("idx", [128, NTB], i32, kind="ExternalInput")
    iog = nc.dram_tensor("iog", [128, 32], i16, kind="ExternalInput")
    identd = nc.dram_tensor("ident", [128, 128], fp16, kind="ExternalInput")
    wk = nc.dram_tensor("wk", [128, KC * E], fp16, kind="ExternalInput")
    biasd = nc.dram_tensor("biasd", [128, EC], f32, kind="ExternalInput")
    out = nc.dram_tensor("out", [E, TOK], fp16, kind="ExternalOutput")

    with tile.TileContext(nc) as tc:
        with (
            tc.tile_pool(name="const", bufs=1) as cpool,
            tc.tile_pool(name="gp",
                         bufs=_POOLCFG.get("gp", GP_BUFS)) as gpool,
            tc.tile_pool(name="cp",
                         bufs=_POOLCFG.get("cp", CP_BUFS)) as cppool,
            tc.tile_pool(name="op", bufs=2) as opool,
            tc.tile_pool(name="ps", bufs=8, space="PSUM") as pspool,
        ):
            idx_all = cpool.tile([128, NTB], i32)
            nc.sync.dma_start(idx_all, idx[:, :])
            iota_s = cpool.tile([128, 32], i16)
            nc.sync.dma_start(iota_s, iog[:, :])
            bias_s = cpool.tile([128, EC], f32)
            nc.sync.dma_start(bias_s, biasd[:, :])
            ident_s = None
            if pe_head:
                ident_s = cpool.tile([128, 128], fp16)
                nc.sync.dma_start(ident_s, identd[:, :])

            # PE p-state warmup + pipeline-fill bridge.
            warmt = cpool.tile([128, 512], fp16)
            nc.scalar.memzero(warmt[:])
            for _ in range(warm):
                pw = pspool.tile([128, 512], f32, name="ps")
                nc.tensor.matmul(pw[:, :], warmt[:, 0:128], warmt[:, :],
                                 start=True, stop=True)

            wk_s = cpool.tile([128, KC * E], fp16)
            g_tiles = {}

            def emit_gather(ci):
                tb0, ntb = chunks[ci]
                G = gpool.tile([128, ntb * CPP], fp16, name="G")
                for t in range(ntb):
                    nc.gpsimd.indirect_dma_start(
                        out=G[:, t * CPP:(t + 1) * CPP],
                        out_offset=None,
                        in_=xb[:, :],
                        in_offset=bass.IndirectOffsetOnAxis(
                            ap=idx_all[:, tb0 + t:tb0 + t + 1], axis=1
                        ),
                    )
                g_tiles[ci] = G

            emit_gather(0)
            emit_gather(1)
            nc.sync.dma_start(wk_s, wk[:, :])

            def prefetch(ci):
                # Keep `lookahead` chunks of row-gathers issued ahead of the
                # (G-dependent) dma_gathers so Pool's in-order queue never
                # stalls the pipeline.
                for cj in range(ci, min(ci + 1 + lookahead, len(chunks))):
                    if cj not in g_tiles:
                        emit_gather(cj)

            for ci, (tb0, ntb) in enumerate(chunks):
                if ci not in g_tiles:
                    emit_gather(ci)
                G = g_tiles.pop(ci)
                st = ntb * 128
                Ct = cppool.tile([128, KC * st], fp16, name="Ct")
                if ci < pe_head and ntb == 1:
                    # Head chunks: transpose on PE slab-by-slab (no dma_gather
                    # latency) and run the matmuls m-outer so the first matmul
                    # starts as soon as slab 0 lands.
                    Cr = Ct[:].rearrange("p (c t) -> p c t", c=KC)
                    outs = opool.tile([128, EC * 512], fp16, name="outs")
                    pss = [pspool.tile([128, st], f32, name="ps")
                           for _ in range(EC)]
                    # Transpose the 6 k-slabs in 2 groups of 3 (one psum bank
                    # + one ACT copy per group) to cut per-slab sem latency.
                    for g in range(2):
                        psT = pspool.tile([128, 384], fp16, name="ps")
                        for j in range(3):
                            m = 3 * g + j
                            nc.tensor.transpose(
                                out=psT[:, j * 128:(j + 1) * 128],
                                in_=G[:, m * 128:(m + 1) * 128],
                                identity=ident_s[:])
                        nc.scalar.activation(
                            Ct[:, g * 384:(g + 1) * 384], psT[:, :],
                            mybir.ActivationFunctionType.Copy)
                        for j in range(3):
                            m = 3 * g + j
                            for e in range(EC):
                                nc.tensor.matmul(
                                    pss[e][:, :],
                                    wk_s[:, m * E + 128 * e:
                                         m * E + 128 * e + 128],
                                    Cr[:, m, :],
                                    start=(m == 0), stop=(m == KC - 1),
                                )
                    for e in range(EC):
                        dst = outs[:, e * st:(e + 1) * st]
                        if e % 2 == 0:
                            nc.scalar.add(dst, pss[e][:, :],
                                          bias_s[:, e:e + 1])
                        else:
                            nc.vector.tensor_scalar_add(dst, pss[e][:, :],
                                                        bias_s[:, e:e + 1])
                    epg = EC // out_halves
                    for h in range(out_halves):
                        ec0 = h * epg
                        nc.sync.dma_start(
                            out[128 * ec0:128 * (ec0 + epg),
                                128 * tb0:128 * tb0 + st].rearrange(
                                    "(ec p) t -> p ec t", ec=epg),
                            outs[:, ec0 * st:(ec0 + epg) * st].rearrange(
                                "p (ec t) -> p ec t", ec=epg),
                        )
                    continue
                if True:
                    nc.gpsimd.dma_gather(
                        out_ap=Ct[:].rearrange("p (c t) -> p c t", c=KC),
                        in_ap=G[:, :],
                        idxs_ap=iota_s[:, 0:st // 16],
                        num_idxs=st,
                        num_idxs_reg=st,
                        elem_size=CPP,
                        transpose=True,
                        sbuf_tokens_per_rank=128,
                        sbuf_free_dim_per_rank=CPP * 2,
                    )
                prefetch(ci + 1)
                oh = out_halves
                Cr = Ct[:].rearrange("p (c t) -> p c t", c=KC)
                outs = opool.tile([128, EC * 512], fp16, name="outs")
                for e in range(EC):
                    ps = pspool.tile([128, st], f32, name="ps")
                    for m in range(KC):
                        nc.tensor.matmul(
                            ps[:, :],
                            wk_s[:, m * E + 128 * e: m * E + 128 * e + 128],
                            Cr[:, m, :],
                            start=(m == 0), stop=(m == KC - 1),
                        )
                    dst = outs[:, e * st:(e + 1) * st]
                    if e % 2 == 0:
                        nc.scalar.add(dst, ps[:, :], bias_s[:, e:e + 1])
                    else:
                        nc.vector.tensor_scalar_add(dst, ps[:, :],
                                                    bias_s[:, e:e + 1])
                epg = EC // oh
                for h in range(oh):
                    ec0 = h * epg
                    nc.sync.dma_start(
                        out[128 * ec0:128 * (ec0 + epg),
                            128 * tb0:128 * tb0 + st].rearrange(
                                "(ec p) t -> p ec t", ec=epg),
                        outs[:, ec0 * st:(ec0 + epg) * st].rearrange(
                            "p (ec t) -> p ec t", ec=epg),
                    )
    nc.finalize()
    return nc


def _host_prep(x, proj_w, proj_b):
    x_hwc = np.ascontiguousarray(
        x.transpose(0, 2, 3, 1)).astype(np.float16)    # [B,H,W,C]
    s = x_hwc.strides
    bands = np.lib.stride_tricks.as_strided(
        x_hwc, shape=(B, NB, H, BW), strides=(s[0], s[2], s[1], s[3]))
    xb = np.ascontiguousarray(bands)                   # [B,369,384,48]

    wk2 = np.ascontiguousarray(
        proj_w.transpose(2, 3, 1, 0).reshape(CPP, E)   # k=(ph,pw,c), E
        .reshape(KC, 128, E).transpose(1, 0, 2).reshape(128, KC * E)
    ).astype(np.float16)
    bias2 = np.ascontiguousarray(
        proj_b.astype(np.float32).reshape(EC, 128).T)  # [128, EC]
    iota = np.empty((128, 32), np.int16)
    for p in range(128):
        iota[p, :] = np.arange(32) * 16 + (p % 16)
    ident = np.eye(128, dtype=np.float16)
    return xb, wk2, bias2, iota, ident


def _prep_core_inputs(xb, start_h, start_w, wk2, bias2, iota, ident, core):
    b0 = core * BPC
    xc = xb[b0:b0 + BPC].reshape(1, -1)
    sh = start_h[b0:b0 + BPC].reshape(TOK).astype(np.int64)
    sw = start_w[b0:b0 + BPC].reshape(TOK).astype(np.int64)
    img = np.repeat(np.arange(BPC, dtype=np.int64), N)
    off = ((img * NB + sw) * H + sh) * BW              # token patch start
    # idx[p, tb] = off[128*tb + p]
    idxT = np.ascontiguousarray(
        off.astype(np.int32).reshape(NTB, 128).T)
    return {"xb": xc, "idx": idxT, "iog": iota, "ident": ident, "wk": wk2,
            "biasd": bias2}


def kernel(x, start_h, start_w, proj_w, proj_b, _run_kwargs=None,
           _return_res=False):
    x = np.asarray(x, dtype=np.float32)
    start_h = np.asarray(start_h, dtype=np.int32)
    start_w = np.asarray(start_w, dtype=np.int32)
    proj_w = np.asarray(proj_w, dtype=np.float32)
    proj_b = np.asarray(proj_b, dtype=np.float32)

    xb, wk2, bias2, iota, ident = _host_prep(x, proj_w, proj_b)

    if "nc" not in _cached:
        _cached["nc"] = build_nc()
    nc = _cached["nc"]

    in_maps = [
        _prep_core_inputs(xb, start_h, start_w, wk2, bias2, iota, ident, c)
        for c in range(NCORES)
    ]
    res = run_bass_kernel_spmd(
        nc, in_maps, core_ids=list(range(NCORES)), **(_run_kwargs or {})
    )
    out = np.concatenate(
        [r["out"].astype(np.float32).T.reshape(BPC, N, E)
         for r in res.results],
        axis=0,
    )
    if _return_res:
        return out, res
    return out


# revision 7
# speedup vs baseline: 1.0018x; 1.0018x over previous
"""Trainium2 Bass kernel for CustomPatchEmbedding — v3.

out[b,n,e] = sum_k patch(b,n)[k] * W[e,k] + bias[e]; patches are 16x16x3
windows of x at (start_h, start_w)[b,n].  8 NeuronCores, data-parallel over
batch (8 images/core, TOK=4608 tokens/core).

Pipeline per chunk of `nt` 128-token blocks:
 1. `nt` indirect row-gathers (one int32 element-offset per partition) land
    token 128*tb+p's patch as 768 contiguous fp16 at G[p, tb-slab] — the
    band layout xb makes every 16x16x3 patch contiguous.
 2. One dma_gather(transpose=True, SBUF source, tokens_per_rank=128) turns
    G into C[p, c, i] = patch element (128c + p) of token i: the matmul-
    ready [k, tok] layout (xbar transposes at 16-bit granularity).
 3. 6x6 weight-stationary fp16 matmuls: psum[e', i] += Wk[128m+p, 128e+e']
    * C[p, m, i], N = 128*nt columns per matmul.  PE does nothing else.
 4. psum -> SBUF drain with per-partition bias (ScalarE activation-Identity
    / VectorE tensor_scalar_add, alternating), then HWDGE DMA to the
    transposed DRAM output [E, TOK]; the host untransposes.

The output is computed transposed so tokens live on the matmul free axis
(weights stationary, loaded 36x per chunk but LDWEIGHTS pipelines under the
moving streams).  Small chunks at the head shorten the pipeline fill;
PE warmup matmuls bridge the fill window so real matmuls run at full
p-state from the first instruction.
"""
import numpy as np

import concourse.bass as bass
import concourse.bacc as bacc
import concourse.mybir as mybir
import concourse.tile as tile
from concourse.bass_utils import run_bass_kernel_spmd

B, C, H, W = 64, 3, 384, 384
N, E, P = 576, 768, 16
NCORES = 8
BPC = B // NCORES          # 8 images per core
TOK = BPC * N              # 4608 tokens per core
CPP = C * P * P            # 768 = contraction length
KC = CPP // 128            # 6 k-chunks
EC = E // 128              # 6 e-chunks
NB = W - P + 1             # 369 one-px-step bands
BW = P * C                 # 48 fp16 per band row
NTB = TOK // 128           # 36 token-blocks of 128
NWARM = 16
GP_BUFS = 8
CP_BUFS = 6
_POOLCFG = {}
CHUNKS = [(0, 1), (1, 1)] + [(2 + 2 * i, 2) for i in range(17)]
assert sum(n for _, n in CHUNKS) == NTB

f32 = mybir.dt.float32
fp16 = mybir.dt.float16
i32 = mybir.dt.int32
i16 = mybir.dt.int16

_cached = {}


def build_nc(debug=False, warm=NWARM, chunks=CHUNKS, out_halves=3,
             lookahead=2, pe_head=1):
    nc = bacc.Bacc(trn_type="TRN2", debug=debug)
    # xb band table: element (b*369+sw)*384*48 + sh*48 + j is
    # x[b, j%3, sh, sw + j//3] (HWC); every patch is 768 contiguous fp16.
    xb = nc.dram_tensor("xb", [1, BPC * NB * H * BW], fp16,
                        kind="ExternalInput")
    idx = nc.dram_tensor("idx", [128, NTB], i32, kind="ExternalInput")
    iog = nc.dram_tensor("iog", [128, 32], i16, kind="ExternalInput")
    identd = nc.dram_tensor("ident", [128, 128], fp16, kind="ExternalInput")
    wk = nc.dram_tensor("wk", [128, KC * E], fp16, kind="ExternalInput")
    biasd = nc.dram_tensor("biasd", [128, EC], f32, kind="ExternalInput")
    out = nc.dram_tensor("out", [E, TOK], fp16, kind="ExternalOutput")

    with tile.TileContext(nc) as tc:
        with (
            tc.tile_pool(name="const", bufs=1) as cpool,
            tc.tile_pool(name="gp",
                         bufs=_POOLCFG.get("gp", GP_BUFS)) as gpool,
            tc.tile_pool(name="cp",
                         bufs=_POOLCFG.get("cp", CP_BUFS)) as cppool,
            tc.tile_pool(name="op", bufs=2) as opool,
            tc.tile_pool(name="ps", bufs=8, space="PSUM") as pspool,
        ):
            idx_all = cpool.tile([128, NTB], i32)
            nc.sync.dma_start(idx_all, idx[:, :])
            iota_s = cpool.tile([128, 32], i16)
            nc.sync.dma_start(iota_s, iog[:, :])
            bias_s = cpool.tile([128, EC], f32)
            nc.sync.dma_start(bias_s, biasd[:, :])
            ident_s = None
            if pe_head:
                ident_s = cpool.tile([128, 128], fp16)
                nc.sync.dma_start(ident_s, identd[:, :])

            # PE p-state warmup + pipeline-fill bridge.
            warmt = cpool.tile([128, 512], fp16)
            nc.scalar.memzero(warmt[:])
            for _ in range(warm):
                pw = pspool.tile([128, 512], f32, name="ps")
                nc.tensor.matmul(pw[:, :], warmt[:, 0:128], warmt[:, :],
                                 start=True, stop=True)

            wk_s = cpool.tile([128, KC * E], fp16)
            g_tiles = {}

            def emit_gather(ci):
                tb0, ntb = chunks[ci]
                G = gpool.tile([128, ntb * CPP], fp16, name="G")
                for t in range(ntb):
                    nc.gpsimd.indirect_dma_start(
                        out=G[:, t * CPP:(t + 1) * CPP],
                        out_offset=None,
                        in_=xb[:, :],
                        in_offset=bass.IndirectOffsetOnAxis(
                            ap=idx_all[:, tb0 + t:tb0 + t + 1], axis=1
                        ),
                    )
                g_tiles[ci] = G

            emit_gather(0)
            emit_gather(1)
            nc.sync.dma_start(wk_s, wk[:, :])

            def prefetch(ci):
                # Keep `lookahead` chunks of row-gathers issued ahead of the
                # (G-dependent) dma_gathers so Pool's in-order queue never
                # stalls the pipeline.
                for cj in range(ci, min(ci + 1 + lookahead, len(chunks))):
                    if cj not in g_tiles:
                        emit_gather(cj)

            for ci, (tb0, ntb) in enumerate(chunks):
                if ci not in g_tiles:
                    emit_gather(ci)
                G = g_tiles.pop(ci)
                st = ntb * 128
                Ct = cppool.tile([128, KC * st], fp16, name="Ct")
                if ci < pe_head and ntb == 1:
                    # Head chunks: transpose on PE slab-by-slab (no dma_gather
                    # latency) and run the matmuls m-outer so the first matmul
                    # starts as soon as slab 0 lands.
                    Cr = Ct[:].rearrange("p (c t) -> p c t", c=KC)
                    outs = opool.tile([128, EC * 512], fp16, name="outs")
                    pss = [pspool.tile([128, st], f32, name="ps")
                           for _ in range(EC)]
                    # Transpose the 6 k-slabs in 2 groups of 3 (one psum bank
                    # + one ACT copy per group) to cut per-slab sem latency.
                    for g in range(2):
                        psT = pspool.tile([128, 384], fp16, name="ps")
                        for j in range(3):
                            m = 3 * g + j
                            nc.tensor.transpose(
                                out=psT[:, j * 128:(j + 1) * 128],
                                in_=G[:, m * 128:(m + 1) * 128],
                                identity=ident_s[:])
                        nc.scalar.activation(
                            Ct[:, g * 384:(g + 1) * 384], psT[:, :],
                            mybir.ActivationFunctionType.Copy)
                        for j in range(3):
                            m = 3 * g + j
                            for e in range(EC):
                                nc.tensor.matmul(
                                    pss[e][:, :],
                                    wk_s[:, m * E + 128 * e:
                                         m * E + 128 * e + 128],
                                    Cr[:, m, :],
                                    start=(m == 0), stop=(m == KC - 1),
                                )
                    for e in range(EC):
                        dst = outs[:, e * st:(e + 1) * st]
                        if e % 2 == 0:
                            nc.scalar.add(dst, pss[e][:, :],
                                          bias_s[:, e:e + 1])
                        else:
                            nc.vector.tensor_scalar_add(dst, pss[e][:, :],
                                                        bias_s[:, e:e + 1])
                    epg = EC // out_halves
                    for h in range(out_halves):
                        ec0 = h * epg
                        nc.sync.dma_start(
                            out[128 * ec0:128 * (ec0 + epg),
                                128 * tb0:128 * tb0 + st].rearrange(
                                    "(ec p) t -> p ec t", ec=epg),
                            outs[:, ec0 * st:(ec0 + epg) * st].rearrange(
                                "p (ec t) -> p ec t", ec=epg),
                        )
                    continue
                if True:
                    nc.gpsimd.dma_gather(
                        out_ap=Ct[:].rearrange("p (c t) -> p c t", c=KC),
                        in_ap=G[:, :],
                        idxs_ap=iota_s[:, 0:st // 16],
                        num_idxs=st,
                        num_idxs_reg=st,
                        elem_size=CPP,
                        transpose=True,
                        sbuf_tokens_per_rank=128,
                        sbuf_free_dim_per_rank=CPP * 2,
                    )
                prefetch(ci + 1)
                oh = out_halves
                Cr = Ct[:].rearrange("p (c t) -> p c t", c=KC)
                outs = opool.tile([128, EC * 512], fp16, name="outs")
                for e in range(EC):
                    ps = pspool.tile([128, st], f32, name="ps")
                    for m in range(KC):
                        nc.tensor.matmul(
                            ps[:, :],
                            wk_s[:, m * E + 128 * e: m * E + 128 * e + 128],
                            Cr[:, m, :],
                            start=(m == 0), stop=(m == KC - 1),
                        )
                    dst = outs[:, e * st:(e + 1) * st]
                    nc.vector.tensor_scalar_add(dst, ps[:, :],
                                                bias_s[:, e:e + 1])
                epg = EC // oh
                for h in range(oh):
                    ec0 = h * epg
                    nc.sync.dma_start(
                        out[128 * ec0:128 * (ec0 + epg),
                            128 * tb0:128 * tb0 + st].rearrange(
                                "(ec p) t -> p ec t", ec=epg),
                        outs[:, ec0 * st:(ec0 + epg) * st].rearrange(
                            "p (ec t) -> p ec t", ec=epg),
                    )
    nc.finalize()
    return nc


def _host_prep(x, proj_w, proj_b):
    x_hwc = np.ascontiguousarray(
        x.transpose(0, 2, 3, 1)).astype(np.float16)    # [B,H,W,C]
    s = x_hwc.strides
    bands = np.lib.stride_tricks.as_strided(
        x_hwc, shape=(B, NB, H, BW), strides=(s[0], s[2], s[1], s[3]))
    xb = np.ascontiguousarray(bands)                   # [B,369,384,48]

    wk2 = np.ascontiguousarray(
        proj_w.transpose(2, 3, 1, 0).reshape(CPP, E)   # k=(ph,pw,c), E
        .reshape(KC, 128, E).transpose(1, 0, 2).reshape(128, KC * E)
    ).astype(np.float16)
    bias2 = np.ascontiguousarray(
        proj_b.astype(np.float32).reshape(EC, 128).T)  # [128, EC]
    iota = np.empty((128, 32), np.int16)
    for p in range(128):
        iota[p, :] = np.arange(32) * 16 + (p % 16)
    ident = np.eye(128, dtype=np.float16)
    return xb, wk2, bias2, iota, ident


def _prep_core_inputs(xb, start_h, start_w, wk2, bias2, iota, ident, core):
    b0 = core * BPC
    xc = xb[b0:b0 + BPC].reshape(1, -1)
    sh = start_h[b0:b0 + BPC].reshape(TOK).astype(np.int64)
    sw = start_w[b0:b0 + BPC].reshape(TOK).astype(np.int64)
    img = np.repeat(np.arange(BPC, dtype=np.int64), N)
    off = ((img * NB + sw) * H + sh) * BW              # token patch start
    # idx[p, tb] = off[128*tb + p]
    idxT = np.ascontiguousarray(
        off.astype(np.int32).reshape(NTB, 128).T)
    return {"xb": xc, "idx": idxT, "iog": iota, "ident": ident, "wk": wk2,
            "biasd": bias2}


def kernel(x, start_h, start_w, proj_w, proj_b, _run_kwargs=None,
           _return_res=False):
    x = np.asarray(x, dtype=np.float32)
    start_h = np.asarray(start_h, dtype=np.int32)
    start_w = np.asarray(start_w, dtype=np.int32)
    proj_w = np.asarray(proj_w, dtype=np.float32)
    proj_b = np.asarray(proj_b, dtype=np.float32)

    xb, wk2, bias2, iota, ident = _host_prep(x, proj_w, proj_b)

    if "nc" not in _cached:
        _cached["nc"] = build_nc()
    nc = _cached["nc"]

    in_maps = [
        _prep_core_inputs(xb, start_h, start_w, wk2, bias2, iota, ident, c)
        for c in range(NCORES)
    ]
    res = run_bass_kernel_spmd(
        nc, in_maps, core_ids=list(range(NCORES)), **(_run_kwargs or {})
    )
    out = np.concatenate(
        [r["out"].astype(np.float32).T.reshape(BPC, N, E)
         for r in res.results],
        axis=0,
    )
    if _return_res:
        return out, res
    return out


# revision 8
# speedup vs baseline: 1.0042x; 1.0023x over previous
"""Trainium2 Bass kernel for CustomPatchEmbedding — v3.

out[b,n,e] = sum_k patch(b,n)[k] * W[e,k] + bias[e]; patches are 16x16x3
windows of x at (start_h, start_w)[b,n].  8 NeuronCores, data-parallel over
batch (8 images/core, TOK=4608 tokens/core).

Pipeline per chunk of `nt` 128-token blocks:
 1. `nt` indirect row-gathers (one int32 element-offset per partition) land
    token 128*tb+p's patch as 768 contiguous fp16 at G[p, tb-slab] — the
    band layout xb makes every 16x16x3 patch contiguous.
 2. One dma_gather(transpose=True, SBUF source, tokens_per_rank=128) turns
    G into C[p, c, i] = patch element (128c + p) of token i: the matmul-
    ready [k, tok] layout (xbar transposes at 16-bit granularity).
 3. 6x6 weight-stationary fp16 matmuls: psum[e', i] += Wk[128m+p, 128e+e']
    * C[p, m, i], N = 128*nt columns per matmul.  PE does nothing else.
 4. psum -> SBUF drain with per-partition bias (ScalarE activation-Identity
    / VectorE tensor_scalar_add, alternating), then HWDGE DMA to the
    transposed DRAM output [E, TOK]; the host untransposes.

The output is computed transposed so tokens live on the matmul free axis
(weights stationary, loaded 36x per chunk but LDWEIGHTS pipelines under the
moving streams).  Small chunks at the head shorten the pipeline fill;
PE warmup matmuls bridge the fill window so real matmuls run at full
p-state from the first instruction.
"""
import numpy as np

import concourse.bass as bass
import concourse.bacc as bacc
import concourse.mybir as mybir
import concourse.tile as tile
from concourse.bass_utils import run_bass_kernel_spmd

B, C, H, W = 64, 3, 384, 384
N, E, P = 576, 768, 16
NCORES = 8
BPC = B // NCORES          # 8 images per core
TOK = BPC * N              # 4608 tokens per core
CPP = C * P * P            # 768 = contraction length
KC = CPP // 128            # 6 k-chunks
EC = E // 128              # 6 e-chunks
NB = W - P + 1             # 369 one-px-step bands
BW = P * C                 # 48 fp16 per band row
NTB = TOK // 128           # 36 token-blocks of 128
NWARM = 16
GP_BUFS = 8
CP_BUFS = 6
_POOLCFG = {}
CHUNKS = [(0, 1), (1, 1)] + [(2 + 2 * i, 2) for i in range(17)]
assert sum(n for _, n in CHUNKS) == NTB

f32 = mybir.dt.float32
fp16 = mybir.dt.float16
i32 = mybir.dt.int32
i16 = mybir.dt.int16

_cached = {}


def build_nc(debug=False, warm=NWARM, chunks=CHUNKS, out_halves=3,
             lookahead=2, pe_head=1):
    nc = bacc.Bacc(trn_type="TRN2", debug=debug)
    # xb band table: element (b*369+sw)*384*48 + sh*48 + j is
    # x[b, j%3, sh, sw + j//3] (HWC); every patch is 768 contiguous fp16.
    xb = nc.dram_tensor("xb", [1, BPC * NB * H * BW], fp16,
                        kind="ExternalInput")
    idx = nc.dram_tensor("idx", [128, NTB], i32, kind="ExternalInput")
    iog = nc.dram_tensor("iog", [128, 32], i16, kind="ExternalInput")
    identd = nc.dram_tensor("ident", [128, 128], fp16, kind="ExternalInput")
    wk = nc.dram_tensor("wk", [128, KC * E], fp16, kind="ExternalInput")
    biasd = nc.dram_tensor("biasd", [128, EC], f32, kind="ExternalInput")
    out = nc.dram_tensor("out", [E, TOK], fp16, kind="ExternalOutput")

    with tile.TileContext(nc) as tc:
        with (
            tc.tile_pool(name="const", bufs=1) as cpool,
            tc.tile_pool(name="gp",
                         bufs=_POOLCFG.get("gp", GP_BUFS)) as gpool,
            tc.tile_pool(name="cp",
                         bufs=_POOLCFG.get("cp", CP_BUFS)) as cppool,
            tc.tile_pool(name="op", bufs=2) as opool,
            tc.tile_pool(name="ps", bufs=8, space="PSUM") as pspool,
        ):
            idx_all = cpool.tile([128, NTB], i32)
            nc.sync.dma_start(idx_all, idx[:, :])
            iota_s = cpool.tile([128, 32], i16)
            nc.sync.dma_start(iota_s, iog[:, :])
            bias_s = cpool.tile([128, EC], f32)
            nc.sync.dma_start(bias_s, biasd[:, :])
            ident_s = None
            if pe_head:
                ident_s = cpool.tile([128, 128], fp16)
                nc.sync.dma_start(ident_s, identd[:, :])

            # PE p-state warmup + pipeline-fill bridge.
            warmt = cpool.tile([128, 512], fp16)
            nc.scalar.memzero(warmt[:])
            for _ in range(warm):
                pw = pspool.tile([128, 512], f32, name="ps")
                nc.tensor.matmul(pw[:, :], warmt[:, 0:128], warmt[:, :],
                                 start=True, stop=True)

            wk_s = cpool.tile([128, KC * E], fp16)
            g_tiles = {}

            def emit_gather(ci):
                tb0, ntb = chunks[ci]
                G = gpool.tile([128, ntb * CPP], fp16, name="G")
                for t in range(ntb):
                    nc.gpsimd.indirect_dma_start(
                        out=G[:, t * CPP:(t + 1) * CPP],
                        out_offset=None,
                        in_=xb[:, :],
                        in_offset=bass.IndirectOffsetOnAxis(
                            ap=idx_all[:, tb0 + t:tb0 + t + 1], axis=1
                        ),
                    )
                g_tiles[ci] = G

            emit_gather(0)
            emit_gather(1)
            nc.sync.dma_start(wk_s, wk[:, :])

            def prefetch(ci):
                # Keep `lookahead` chunks of row-gathers issued ahead of the
                # (G-dependent) dma_gathers so Pool's in-order queue never
                # stalls the pipeline.
                for cj in range(ci, min(ci + 1 + lookahead, len(chunks))):
                    if cj not in g_tiles:
                        emit_gather(cj)

            for ci, (tb0, ntb) in enumerate(chunks):
                if ci not in g_tiles:
                    emit_gather(ci)
                G = g_tiles.pop(ci)
                st = ntb * 128
                Ct = cppool.tile([128, KC * st], fp16, name="Ct")
                if ci < pe_head and ntb == 1:
                    # Head chunks: transpose on PE slab-by-slab (no dma_gather
                    # latency) and run the matmuls m-outer so the first matmul
                    # starts as soon as slab 0 lands.
                    Cr = Ct[:].rearrange("p (c t) -> p c t", c=KC)
                    outs = opool.tile([128, EC * 512], fp16, name="outs")
                    pss = [pspool.tile([128, st], f32, name="ps")
                           for _ in range(EC)]
                    # Transpose the 6 k-slabs in 3 groups of 2 (one psum bank
                    # + one ACT copy per group) to cut per-slab sem latency.
                    for g in range(3):
                        psT = pspool.tile([128, 256], fp16, name="ps")
                        for j in range(2):
                            m = 2 * g + j
                            nc.tensor.transpose(
                                out=psT[:, j * 128:(j + 1) * 128],
                                in_=G[:, m * 128:(m + 1) * 128],
                                identity=ident_s[:])
                        nc.scalar.activation(
                            Ct[:, g * 256:(g + 1) * 256], psT[:, :],
                            mybir.ActivationFunctionType.Copy)
                        for j in range(2):
                            m = 2 * g + j
                            for e in range(EC):
                                nc.tensor.matmul(
                                    pss[e][:, :],
                                    wk_s[:, m * E + 128 * e:
                                         m * E + 128 * e + 128],
                                    Cr[:, m, :],
                                    start=(m == 0), stop=(m == KC - 1),
                                )
                    for e in range(EC):
                        dst = outs[:, e * st:(e + 1) * st]
                        if e % 2 == 0:
                            nc.scalar.add(dst, pss[e][:, :],
                                          bias_s[:, e:e + 1])
                        else:
                            nc.vector.tensor_scalar_add(dst, pss[e][:, :],
                                                        bias_s[:, e:e + 1])
                    epg = EC // out_halves
                    for h in range(out_halves):
                        ec0 = h * epg
                        nc.sync.dma_start(
                            out[128 * ec0:128 * (ec0 + epg),
                                128 * tb0:128 * tb0 + st].rearrange(
                                    "(ec p) t -> p ec t", ec=epg),
                            outs[:, ec0 * st:(ec0 + epg) * st].rearrange(
                                "p (ec t) -> p ec t", ec=epg),
                        )
                    continue
                if True:
                    nc.gpsimd.dma_gather(
                        out_ap=Ct[:].rearrange("p (c t) -> p c t", c=KC),
                        in_ap=G[:, :],
                        idxs_ap=iota_s[:, 0:st // 16],
                        num_idxs=st,
                        num_idxs_reg=st,
                        elem_size=CPP,
                        transpose=True,
                        sbuf_tokens_per_rank=128,
                        sbuf_free_dim_per_rank=CPP * 2,
                    )
                prefetch(ci + 1)
                oh = out_halves
                Cr = Ct[:].rearrange("p (c t) -> p c t", c=KC)
                outs = opool.tile([128, EC * 512], fp16, name="outs")
                for e in range(EC):
                    ps = pspool.tile([128, st], f32, name="ps")
                    for m in range(KC):
                        nc.tensor.matmul(
                            ps[:, :],
                            wk_s[:, m * E + 128 * e: m * E + 128 * e + 128],
                            Cr[:, m, :],
                            start=(m == 0), stop=(m == KC - 1),
                        )
                    dst = outs[:, e * st:(e + 1) * st]
                    nc.vector.tensor_scalar_add(dst, ps[:, :],
                                                bias_s[:, e:e + 1])
                epg = EC // oh
                for h in range(oh):
                    ec0 = h * epg
                    nc.sync.dma_start(
                        out[128 * ec0:128 * (ec0 + epg),
                            128 * tb0:128 * tb0 + st].rearrange(
                                "(ec p) t -> p ec t", ec=epg),
                        outs[:, ec0 * st:(ec0 + epg) * st].rearrange(
                            "p (ec t) -> p ec t", ec=epg),
                    )
    nc.finalize()
    return nc


def _host_prep(x, proj_w, proj_b):
    x_hwc = np.ascontiguousarray(
        x.transpose(0, 2, 3, 1)).astype(np.float16)    # [B,H,W,C]
    s = x_hwc.strides
    bands = np.lib.stride_tricks.as_strided(
        x_hwc, shape=(B, NB, H, BW), strides=(s[0], s[2], s[1], s[3]))
    xb = np.ascontiguousarray(bands)                   # [B,369,384,48]

    wk2 = np.ascontiguousarray(
        proj_w.transpose(2, 3, 1, 0).reshape(CPP, E)   # k=(ph,pw,c), E
        .reshape(KC, 128, E).transpose(1, 0, 2).reshape(128, KC * E)
    ).astype(np.float16)
    bias2 = np.ascontiguousarray(
        proj_b.astype(np.float32).reshape(EC, 128).T)  # [128, EC]
    iota = np.empty((128, 32), np.int16)
    for p in range(128):
        iota[p, :] = np.arange(32) * 16 + (p % 16)
    ident = np.eye(128, dtype=np.float16)
    return xb, wk2, bias2, iota, ident


def _prep_core_inputs(xb, start_h, start_w, wk2, bias2, iota, ident, core):
    b0 = core * BPC
    xc = xb[b0:b0 + BPC].reshape(1, -1)
    sh = start_h[b0:b0 + BPC].reshape(TOK).astype(np.int64)
    sw = start_w[b0:b0 + BPC].reshape(TOK).astype(np.int64)
    img = np.repeat(np.arange(BPC, dtype=np.int64), N)
    off = ((img * NB + sw) * H + sh) * BW              # token patch start
    # idx[p, tb] = off[128*tb + p]
    idxT = np.ascontiguousarray(
        off.astype(np.int32).reshape(NTB, 128).T)
    return {"xb": xc, "idx": idxT, "iog": iota, "ident": ident, "wk": wk2,
            "biasd": bias2}


def kernel(x, start_h, start_w, proj_w, proj_b, _run_kwargs=None,
           _return_res=False):
    x = np.asarray(x, dtype=np.float32)
    start_h = np.asarray(start_h, dtype=np.int32)
    start_w = np.asarray(start_w, dtype=np.int32)
    proj_w = np.asarray(proj_w, dtype=np.float32)
    proj_b = np.asarray(proj_b, dtype=np.float32)

    xb, wk2, bias2, iota, ident = _host_prep(x, proj_w, proj_b)

    if "nc" not in _cached:
        _cached["nc"] = build_nc()
    nc = _cached["nc"]

    in_maps = [
        _prep_core_inputs(xb, start_h, start_w, wk2, bias2, iota, ident, c)
        for c in range(NCORES)
    ]
    res = run_bass_kernel_spmd(
        nc, in_maps, core_ids=list(range(NCORES)), **(_run_kwargs or {})
    )
    out = np.concatenate(
        [r["out"].astype(np.float32).T.reshape(BPC, N, E)
         for r in res.results],
        axis=0,
    )
    if _return_res:
        return out, res
    return out


# revision 9
# speedup vs baseline: 1.0131x; 1.0089x over previous
"""Trainium2 Bass kernel for CustomPatchEmbedding — v3.

out[b,n,e] = sum_k patch(b,n)[k] * W[e,k] + bias[e]; patches are 16x16x3
windows of x at (start_h, start_w)[b,n].  8 NeuronCores, data-parallel over
batch (8 images/core, TOK=4608 tokens/core).

Pipeline per chunk of `nt` 128-token blocks:
 1. `nt` indirect row-gathers (one int32 element-offset per partition) land
    token 128*tb+p's patch as 768 contiguous fp16 at G[p, tb-slab] — the
    band layout xb makes every 16x16x3 patch contiguous.
 2. One dma_gather(transpose=True, SBUF source, tokens_per_rank=128) turns
    G into C[p, c, i] = patch element (128c + p) of token i: the matmul-
    ready [k, tok] layout (xbar transposes at 16-bit granularity).
 3. 6x6 weight-stationary fp16 matmuls: psum[e', i] += Wk[128m+p, 128e+e']
    * C[p, m, i], N = 128*nt columns per matmul.  PE does nothing else.
 4. psum -> SBUF drain with per-partition bias (ScalarE activation-Identity
    / VectorE tensor_scalar_add, alternating), then HWDGE DMA to the
    transposed DRAM output [E, TOK]; the host untransposes.

The output is computed transposed so tokens live on the matmul free axis
(weights stationary, loaded 36x per chunk but LDWEIGHTS pipelines under the
moving streams).  Small chunks at the head shorten the pipeline fill;
PE warmup matmuls bridge the fill window so real matmuls run at full
p-state from the first instruction.
"""
import numpy as np

import concourse.bass as bass
import concourse.bacc as bacc
import concourse.mybir as mybir
import concourse.tile as tile
from concourse.bass_utils import run_bass_kernel_spmd

B, C, H, W = 64, 3, 384, 384
N, E, P = 576, 768, 16
NCORES = 8
BPC = B // NCORES          # 8 images per core
TOK = BPC * N              # 4608 tokens per core
CPP = C * P * P            # 768 = contraction length
KC = CPP // 128            # 6 k-chunks
EC = E // 128              # 6 e-chunks
NB = W - P + 1             # 369 one-px-step bands
BW = P * C                 # 48 fp16 per band row
NTB = TOK // 128           # 36 token-blocks of 128
NWARM = 16
GP_BUFS = 8
CP_BUFS = 6
_POOLCFG = {}
CHUNKS = [(0, 1), (1, 1)] + [(2 + 2 * i, 2) for i in range(17)]
assert sum(n for _, n in CHUNKS) == NTB

f32 = mybir.dt.float32
fp16 = mybir.dt.float16
i32 = mybir.dt.int32
i16 = mybir.dt.int16

_cached = {}


def build_nc(debug=False, warm=NWARM, chunks=CHUNKS, out_halves=3,
             lookahead=1, pe_head=1):
    nc = bacc.Bacc(trn_type="TRN2", debug=debug)
    # xb band table: element (b*369+sw)*384*48 + sh*48 + j is
    # x[b, j%3, sh, sw + j//3] (HWC); every patch is 768 contiguous fp16.
    xb = nc.dram_tensor("xb", [1, BPC * NB * H * BW], fp16,
                        kind="ExternalInput")
    idx = nc.dram_tensor("idx", [128, NTB], i32, kind="ExternalInput")
    iog = nc.dram_tensor("iog", [128, 32], i16, kind="ExternalInput")
    identd = nc.dram_tensor("ident", [128, 128], fp16, kind="ExternalInput")
    wk = nc.dram_tensor("wk", [128, KC * E], fp16, kind="ExternalInput")
    biasd = nc.dram_tensor("biasd", [128, EC], f32, kind="ExternalInput")
    out = nc.dram_tensor("out", [E, TOK], fp16, kind="ExternalOutput")

    with tile.TileContext(nc) as tc:
        with (
            tc.tile_pool(name="const", bufs=1) as cpool,
            tc.tile_pool(name="gp",
                         bufs=_POOLCFG.get("gp", GP_BUFS)) as gpool,
            tc.tile_pool(name="cp",
                         bufs=_POOLCFG.get("cp", CP_BUFS)) as cppool,
            tc.tile_pool(name="op", bufs=2) as opool,
            tc.tile_pool(name="ps", bufs=8, space="PSUM") as pspool,
        ):
            idx_all = cpool.tile([128, NTB], i32)
            nc.sync.dma_start(idx_all, idx[:, :])
            iota_s = cpool.tile([128, 32], i16)
            nc.sync.dma_start(iota_s, iog[:, :])
            bias_s = cpool.tile([128, EC], f32)
            nc.sync.dma_start(bias_s, biasd[:, :])
            ident_s = None
            if pe_head:
                ident_s = cpool.tile([128, 128], fp16)
                nc.sync.dma_start(ident_s, identd[:, :])

            # PE p-state warmup + pipeline-fill bridge.
            warmt = cpool.tile([128, 512], fp16)
            nc.scalar.memzero(warmt[:])
            for _ in range(warm):
                pw = pspool.tile([128, 512], f32, name="ps")
                nc.tensor.matmul(pw[:, :], warmt[:, 0:128], warmt[:, :],
                                 start=True, stop=True)

            wk_s = cpool.tile([128, KC * E], fp16)
            g_tiles = {}

            def emit_gather(ci):
                tb0, ntb = chunks[ci]
                G = gpool.tile([128, ntb * CPP], fp16, name="G")
                for t in range(ntb):
                    nc.gpsimd.indirect_dma_start(
                        out=G[:, t * CPP:(t + 1) * CPP],
                        out_offset=None,
                        in_=xb[:, :],
                        in_offset=bass.IndirectOffsetOnAxis(
                            ap=idx_all[:, tb0 + t:tb0 + t + 1], axis=1
                        ),
                    )
                g_tiles[ci] = G

            emit_gather(0)
            emit_gather(1)
            nc.sync.dma_start(wk_s, wk[:, :])

            def prefetch(ci):
                # Keep `lookahead` chunks of row-gathers issued ahead of the
                # (G-dependent) dma_gathers so Pool's in-order queue never
                # stalls the pipeline.
                for cj in range(ci, min(ci + 1 + lookahead, len(chunks))):
                    if cj not in g_tiles:
                        emit_gather(cj)

            for ci, (tb0, ntb) in enumerate(chunks):
                if ci not in g_tiles:
                    emit_gather(ci)
                G = g_tiles.pop(ci)
                st = ntb * 128
                Ct = cppool.tile([128, KC * st], fp16, name="Ct")
                if ci < pe_head and ntb == 1:
                    # Head chunks: transpose on PE slab-by-slab (no dma_gather
                    # latency) and run the matmuls m-outer so the first matmul
                    # starts as soon as slab 0 lands.
                    Cr = Ct[:].rearrange("p (c t) -> p c t", c=KC)
                    outs = opool.tile([128, EC * 512], fp16, name="outs")
                    pss = [pspool.tile([128, st], f32, name="ps")
                           for _ in range(EC)]
                    # Transpose the 6 k-slabs in 3 groups of 2 (one psum bank
                    # + one ACT copy per group) to cut per-slab sem latency.
                    for g in range(3):
                        psT = pspool.tile([128, 256], fp16, name="ps")
                        for j in range(2):
                            m = 2 * g + j
                            nc.tensor.transpose(
                                out=psT[:, j * 128:(j + 1) * 128],
                                in_=G[:, m * 128:(m + 1) * 128],
                                identity=ident_s[:])
                        nc.scalar.activation(
                            Ct[:, g * 256:(g + 1) * 256], psT[:, :],
                            mybir.ActivationFunctionType.Copy)
                        for j in range(2):
                            m = 2 * g + j
                            for e in range(EC):
                                nc.tensor.matmul(
                                    pss[e][:, :],
                                    wk_s[:, m * E + 128 * e:
                                         m * E + 128 * e + 128],
                                    Cr[:, m, :],
                                    start=(m == 0), stop=(m == KC - 1),
                                )
                    for e in range(EC):
                        dst = outs[:, e * st:(e + 1) * st]
                        if e % 2 == 0:
                            nc.scalar.add(dst, pss[e][:, :],
                                          bias_s[:, e:e + 1])
                        else:
                            nc.vector.tensor_scalar_add(dst, pss[e][:, :],
                                                        bias_s[:, e:e + 1])
                    epg = EC // out_halves
                    for h in range(out_halves):
                        ec0 = h * epg
                        nc.sync.dma_start(
                            out[128 * ec0:128 * (ec0 + epg),
                                128 * tb0:128 * tb0 + st].rearrange(
                                    "(ec p) t -> p ec t", ec=epg),
                            outs[:, ec0 * st:(ec0 + epg) * st].rearrange(
                                "p (ec t) -> p ec t", ec=epg),
                        )
                    continue
                if True:
                    nc.gpsimd.dma_gather(
                        out_ap=Ct[:].rearrange("p (c t) -> p c t", c=KC),
                        in_ap=G[:, :],
                        idxs_ap=iota_s[:, 0:st // 16],
                        num_idxs=st,
                        num_idxs_reg=st,
                        elem_size=CPP,
                        transpose=True,
                        sbuf_tokens_per_rank=128,
                        sbuf_free_dim_per_rank=CPP * 2,
                    )
                prefetch(ci + 1)
                oh = out_halves
                Cr = Ct[:].rearrange("p (c t) -> p c t", c=KC)
                outs = opool.tile([128, EC * 512], fp16, name="outs")
                for e in range(EC):
                    ps = pspool.tile([128, st], f32, name="ps")
                    for m in range(KC):
                        nc.tensor.matmul(
                            ps[:, :],
                            wk_s[:, m * E + 128 * e: m * E + 128 * e + 128],
                            Cr[:, m, :],
                            start=(m == 0), stop=(m == KC - 1),
                        )
                    dst = outs[:, e * st:(e + 1) * st]
                    nc.vector.tensor_scalar_add(dst, ps[:, :],
                                                bias_s[:, e:e + 1])
                epg = EC // oh
                for h in range(oh):
                    ec0 = h * epg
                    nc.sync.dma_start(
                        out[128 * ec0:128 * (ec0 + epg),
                            128 * tb0:128 * tb0 + st].rearrange(
                                "(ec p) t -> p ec t", ec=epg),
                        outs[:, ec0 * st:(ec0 + epg) * st].rearrange(
                            "p (ec t) -> p ec t", ec=epg),
                    )
    nc.finalize()
    return nc


def _host_prep(x, proj_w, proj_b):
    x_hwc = np.ascontiguousarray(
        x.transpose(0, 2, 3, 1)).astype(np.float16)    # [B,H,W,C]
    s = x_hwc.strides
    bands = np.lib.stride_tricks.as_strided(
        x_hwc, shape=(B, NB, H, BW), strides=(s[0], s[2], s[1], s[3]))
    xb = np.ascontiguousarray(bands)                   # [B,369,384,48]

    wk2 = np.ascontiguousarray(
        proj_w.transpose(2, 3, 1, 0).reshape(CPP, E)   # k=(ph,pw,c), E
        .reshape(KC, 128, E).transpose(1, 0, 2).reshape(128, KC * E)
    ).astype(np.float16)
    bias2 = np.ascontiguousarray(
        proj_b.astype(np.float32).reshape(EC, 128).T)  # [128, EC]
    iota = np.empty((128, 32), np.int16)
    for p in range(128):
        iota[p, :] = np.arange(32) * 16 + (p % 16)
    ident = np.eye(128, dtype=np.float16)
    return xb, wk2, bias2, iota, ident


def _prep_core_inputs(xb, start_h, start_w, wk2, bias2, iota, ident, core):
    b0 = core * BPC
    xc = xb[b0:b0 + BPC].reshape(1, -1)
    sh = start_h[b0:b0 + BPC].reshape(TOK).astype(np.int64)
    sw = start_w[b0:b0 + BPC].reshape(TOK).astype(np.int64)
    img = np.repeat(np.arange(BPC, dtype=np.int64), N)
    off = ((img * NB + sw) * H + sh) * BW              # token patch start
    # idx[p, tb] = off[128*tb + p]
    idxT = np.ascontiguousarray(
        off.astype(np.int32).reshape(NTB, 128).T)
    return {"xb": xc, "idx": idxT, "iog": iota, "ident": ident, "wk": wk2,
            "biasd": bias2}


def kernel(x, start_h, start_w, proj_w, proj_b, _run_kwargs=None,
           _return_res=False):
    x = np.asarray(x, dtype=np.float32)
    start_h = np.asarray(start_h, dtype=np.int32)
    start_w = np.asarray(start_w, dtype=np.int32)
    proj_w = np.asarray(proj_w, dtype=np.float32)
    proj_b = np.asarray(proj_b, dtype=np.float32)

    xb, wk2, bias2, iota, ident = _host_prep(x, proj_w, proj_b)

    if "nc" not in _cached:
        _cached["nc"] = build_nc()
    nc = _cached["nc"]

    in_maps = [
        _prep_core_inputs(xb, start_h, start_w, wk2, bias2, iota, ident, c)
        for c in range(NCORES)
    ]
    res = run_bass_kernel_spmd(
        nc, in_maps, core_ids=list(range(NCORES)), **(_run_kwargs or {})
    )
    out = np.concatenate(
        [r["out"].astype(np.float32).T.reshape(BPC, N, E)
         for r in res.results],
        axis=0,
    )
    if _return_res:
        return out, res
    return out
